# revision 1
# baseline (speedup 1.0000x reference)
"""Trainium2 Bass kernel for nn_BERTSyntaxRel (biaffine syntax-relation head).

Computation (per batch b, token t):
    appended = concat([root, x[b]])                      # (S+1, D)
    gathered = appended[head_id[b, t]]                   # (D,)
    head = relu(gathered @ Wh + bh)                      # (H,)
    tail = relu(x[b, t] @ Wt + bt)                       # (H,)
    out[b, t, r] = sum_{h,k} head[h] * K[h, r, k] * tail[k]

Sharding: data-parallel over batch, 4 batches per core on 8 cores.
Weights replicated.  No collectives needed.

Key restructure: the gather commutes with the row-wise head FF, so we
compute head_all = relu([root; x[b]] @ Wh + bh) for ALL positions first
(same FLOPs), write the (1025, H) per-batch table to DRAM, and gather
H=128-float rows instead of D=768-float x rows.
"""

import numpy as np

B, S, D, H, R = 32, 1024, 768, 128, 48
NCORES = 8
BPC = B // NCORES            # batches per core (4)
TOK = BPC * S                # tokens per core (4096)
P = 128                      # partition dim / token tile
NTILES = TOK // P            # 32 token tiles per core
TBL = S + 1                  # rows per batch gather table (1025)
DC = D // P                  # 6 contraction chunks of 128
RKCH = 12                    # biaffine free-dim chunks of 512 (R*H = 6144)
RPC = 4                      # r values per 512-chunk
import os as _os

NDVE = int(_os.environ.get("K_NDVE", "4"))  # chunks via DVE mul+reduce;
                             # the rest go ACT-copy -> gpsimd-mul -> DVE-reduce
DEPEDGE = _os.environ.get("K_DEPEDGE", "1") == "1"  # explicit gather->table-write deps
ILV = _os.environ.get("K_ILV", "1") == "1"  # interleave Phase A/B emission per batch
STT = _os.environ.get("K_STT", "0") == "1"  # fused scalar_tensor_tensor consume on DVE
PAIR = _os.environ.get("K_PAIR", "0") == "1"  # 2-bank (P,1024) consume chunks
MERGEAF = _os.environ.get("K_MERGEAF", "1") == "1"  # share psA/psF slots, psM=5
HTA = _os.environ.get("K_HTA", "0") == "1"  # gather-transpose PSUM from the A pool


def build_program(with_bias=True):
    """Build the Bass program (shared by all 8 cores, SPMD)."""
    from contextlib import ExitStack

    import concourse.bass as bass
    import concourse.tile as tile
    from concourse import bacc, mybir
    from concourse.masks import make_identity

    f32 = mybir.dt.float32
    i32 = mybir.dt.int32
    ts = bass.ts

    nc = bacc.Bacc(
        "TRN2",
        target_bir_lowering=False,
        debug=False,
        num_devices=NCORES,
    )

    x_ap = nc.dram_tensor("x", [TOK, D], f32, kind="ExternalInput").ap()
    gidx_ap = nc.dram_tensor("gidx", [TOK, 1], i32, kind="ExternalInput").ap()
    wh_ap = nc.dram_tensor("Wh", [D, H], f32, kind="ExternalInput").ap()
    wt_ap = nc.dram_tensor("Wt", [D, H], f32, kind="ExternalInput").ap()
    bh_ap = nc.dram_tensor("bh", [1, H], f32, kind="ExternalInput").ap()
    bt_ap = nc.dram_tensor("bt", [1, H], f32, kind="ExternalInput").ap()
    rooth_ap = nc.dram_tensor("rooth", [1, H], f32, kind="ExternalInput").ap()
    kern_ap = nc.dram_tensor("kern", [H, R * H], f32, kind="ExternalInput").ap()
    out_ap = nc.dram_tensor("out", [TOK, R], f32, kind="ExternalOutput").ap()

    with tile.TileContext(nc) as tc, ExitStack() as ctx:
        # ---- constants / weights, resident for the whole kernel ----
        const = ctx.enter_context(tc.tile_pool(name="const", bufs=1))
        ident = const.tile([P, P], f32)
        make_identity(nc, ident[:])
        ones1 = const.tile([1, P], f32)
        nc.gpsimd.memset(ones1[:], 1.0)
        # combined FF weights: per d-chunk c, wht[:, c*256 : c*256+128] = Wh chunk,
        # wht[:, c*256+128 : (c+1)*256] = Wt chunk -> one N=256 matmul per chunk
        wht = const.tile([P, 2 * D], f32)
        wh3 = wh_ap.rearrange("(c p) h -> c p h", p=P)
        wt3 = wt_ap.rearrange("(c p) h -> c p h", p=P)
        for c in range(DC):
            nc.sync.dma_start(out=wht[:, ts(2 * c, P)], in_=wh3[c])
            nc.sync.dma_start(out=wht[:, ts(2 * c + 1, P)], in_=wt3[c])
        bb_sb = const.tile([1, 2 * H], f32)
        rt_sb = const.tile([1, H], f32)
        nc.sync.dma_start(out=bb_sb[:, :H], in_=bh_ap[:])
        nc.sync.dma_start(out=bb_sb[:, H:], in_=bt_ap[:])
        nc.sync.dma_start(out=rt_sb[:], in_=rooth_ap[:])
        ksb = const.tile([H, R * H], f32)  # 24KB/partition
        nc.sync.dma_start(out=ksb[:], in_=kern_ap[:])

        # tail (tok-major) for the whole core, kept in SBUF: 16KB/partition
        tailT_all = const.tile([P, TOK], f32)

        # per-batch gather tables in DRAM: row b*TBL is the root head state
        dram = ctx.enter_context(tc.tile_pool(name="dram", bufs=1, space="DRAM"))
        head_all = dram.tile([BPC * TBL, H], f32)
        # head_all writers per batch (Tile does not track DRAM deps; the
        # Phase-B gathers get explicit dep edges on these)
        tbl_writes = [[] for _ in range(BPC)]
        for b in range(BPC):
            w = nc.sync.dma_start(
                out=head_all[b * TBL : b * TBL + 1, :], in_=rt_sb[:1, :]
            )
            tbl_writes[b].append(w.ins)

        # ---- Phase A: transposes + FFs; fills head_all (DRAM) and tailT_all ----
        def emit_A(i, xa_pool, xt_pool, ha_pool, psA, psF):
                b = i // (S // P)
                xt = xa_pool.tile([P, D], f32)
                nc.sync.dma_start(out=xt[:], in_=x_ap[ts(i, P), :])
                xT = xt_pool.tile([P, D], f32)
                if ILV:
                    # 1-bank PSUM tiles so Phase A+B pools fit in 8 banks
                    for half in range(2):
                        xT_ps = psA.tile([P, D // 2], f32, tag="psa")
                        for c3 in range(DC // 2):
                            c = half * (DC // 2) + c3
                            nc.tensor.transpose(
                                out=xT_ps[:, ts(c3, P)], in_=xt[:, ts(c, P)],
                                identity=ident[:],
                            )
                        nc.scalar.copy(out=xT[:, ts(half, D // 2)], in_=xT_ps[:])
                else:
                    xT_ps = psA.tile([P, D], f32)
                    for c in range(DC):
                        nc.tensor.transpose(
                            out=xT_ps[:, ts(c, P)], in_=xt[:, ts(c, P)],
                            identity=ident[:],
                        )
                    nc.scalar.copy(out=xT[:], in_=xT_ps[:])

                ff_ps = psF.tile([P, 2 * H], f32, tag="psa" if (ILV and psF is psA) else "ff_ps")
                for c in range(DC):
                    nc.tensor.matmul(
                        out=ff_ps[:], lhsT=xT[:, ts(c, P)], rhs=wht[:, ts(c, 2 * P)],
                        start=(c == 0), stop=(c == DC - 1 and not with_bias),
                    )
                if with_bias:
                    nc.tensor.matmul(
                        out=ff_ps[:], lhsT=ones1[:1, :], rhs=bb_sb[:1, :],
                        start=False, stop=True,
                    )
                hA = ha_pool.tile([P, H], f32)
                nc.scalar.activation(
                    out=hA[:], in_=ff_ps[:, :H], func=mybir.ActivationFunctionType.Relu
                )
                nc.scalar.activation(
                    out=tailT_all[:, ts(i, P)], in_=ff_ps[:, H:],
                    func=mybir.ActivationFunctionType.Relu,
                )
                row0 = b * TBL + 1 + (i % (S // P)) * P
                w = nc.sync.dma_start(out=head_all[row0 : row0 + P, :], in_=hA[:])
                tbl_writes[b].append(w.ins)

        # ---- Phase B: gather + biaffine ----
        def emit_B(i, gx_pool, gb_pool, hb_pool, prod_pool, ob_pool, psT, psM):
                b = i // (S // P)
                gix = gx_pool.tile([P, 1], i32)
                nc.sync.dma_start(out=gix[:], in_=gidx_ap[ts(i, P), :])
                g_sb = gb_pool.tile([P, H], f32)
                g = nc.gpsimd.indirect_dma_start(
                    out=g_sb[:],
                    out_offset=None,
                    in_=head_all[:],
                    in_offset=bass.IndirectOffsetOnAxis(ap=gix[:, :1], axis=0),
                )
                if DEPEDGE or ILV:  # mandatory when there is no phase barrier
                    for w_ins in tbl_writes[b]:
                        tile.add_dep_helper(
                            g.ins, w_ins, sync=True, reason="head_all RAW"
                        )
                hT_tag = "psa" if (ILV and psT is not psM) else (
                    "tmp_ps" if psT is psM else "hT_ps"
                )
                hT_ps = psT.tile([P, H], f32, tag=hT_tag)
                nc.tensor.transpose(out=hT_ps[:], in_=g_sb[:], identity=ident[:])
                head_sb = hb_pool.tile([P, H], f32)
                nc.scalar.copy(out=head_sb[:], in_=hT_ps[:])

                out_sb = ob_pool.tile([P, R], f32)
                tlT = tailT_all[:, ts(i, P)]
                tl3 = tlT.rearrange("p (o k) -> p o k", o=1).to_broadcast([P, RPC, H])
                if PAIR:
                    # paired 2-bank chunks: halve the per-op overhead on the
                    # consume engines (DVE/ACT/gpsimd ops are 1024 wide)
                    tl8 = tlT.rearrange("p (o k) -> p o k", o=1).to_broadcast(
                        [P, 2 * RPC, H]
                    )
                    for jp in range(RKCH // 2):
                        tmp2 = psM.tile([P, 1024], f32, tag="tmp_ps")
                        for h2 in range(2):
                            nc.tensor.matmul(
                                out=tmp2[:, ts(h2, 512)],
                                lhsT=head_sb[:],
                                rhs=ksb[:, ts(2 * jp + h2, 512)],
                                start=True, stop=True,
                            )
                        if jp < (NDVE + 1) // 2:
                            prod = prod_pool.tile([P, 1024], f32, tag="pr0")
                            nc.vector.tensor_tensor(
                                out=prod[:].rearrange("p (r k) -> p r k", k=H),
                                in0=tmp2[:].rearrange("p (r k) -> p r k", k=H),
                                in1=tl8,
                                op=mybir.AluOpType.mult,
                            )
                            nc.vector.tensor_reduce(
                                out=out_sb[:, ts(jp, 2 * RPC)],
                                in_=prod[:].rearrange("p (r k) -> p r k", k=H),
                                axis=mybir.AxisListType.X,
                                op=mybir.AluOpType.add,
                            )
                        else:
                            cp = prod_pool.tile([P, 1024], f32, tag="cp")
                            nc.scalar.copy(out=cp[:], in_=tmp2[:])
                            pr = prod_pool.tile([P, 1024], f32, tag="pr")
                            nc.gpsimd.tensor_tensor(
                                out=pr[:].rearrange("p (r k) -> p r k", k=H),
                                in0=cp[:].rearrange("p (r k) -> p r k", k=H),
                                in1=tl8,
                                op=mybir.AluOpType.mult,
                            )
                            nc.vector.tensor_reduce(
                                out=out_sb[:, ts(jp, 2 * RPC)],
                                in_=pr[:].rearrange("p (r k) -> p r k", k=H),
                                axis=mybir.AxisListType.X,
                                op=mybir.AluOpType.add,
                            )
                    nc.sync.dma_start(out=out_ap[ts(i, P), :], in_=out_sb[:])
                    return
                for j in range(RKCH):
                    tmp_ps = psM.tile([P, 512], f32)
                    nc.tensor.matmul(
                        out=tmp_ps[:], lhsT=head_sb[:], rhs=ksb[:, ts(j, 512)],
                        start=True, stop=True,
                    )
                    if j < NDVE:
                        if STT:
                            # fused (tmp * tailT) + free-dim accum per r on DVE
                            scr = prod_pool.tile([P, 512], f32, tag="pr0")
                            for q in range(RPC):
                                r = j * RPC + q
                                nc.vector.scalar_tensor_tensor(
                                    out=scr[:, ts(q, H)],
                                    in0=tmp_ps[:, ts(q, H)],
                                    scalar=1.0,
                                    in1=tlT,
                                    op0=mybir.AluOpType.mult,
                                    op1=mybir.AluOpType.mult,
                                    accum_out=out_sb[:, r : r + 1],
                                )
                        else:
                            # DVE: tensor_tensor mul (reads tmp from PSUM) + reduce
                            prod = prod_pool.tile([P, 512], f32, tag="pr0")
                            nc.vector.tensor_tensor(
                                out=prod[:].rearrange("p (r k) -> p r k", k=H),
                                in0=tmp_ps[:].rearrange("p (r k) -> p r k", k=H),
                                in1=tl3,
                                op=mybir.AluOpType.mult,
                            )
                            nc.vector.tensor_reduce(
                                out=out_sb[:, ts(j, RPC)],
                                in_=prod[:].rearrange("p (r k) -> p r k", k=H),
                                axis=mybir.AxisListType.X,
                                op=mybir.AluOpType.add,
                            )
                    else:
                        # ACT evacuates PSUM, gpsimd multiplies, DVE reduces
                        cp = prod_pool.tile([P, 512], f32, tag="cp")
                        nc.scalar.copy(out=cp[:], in_=tmp_ps[:])
                        pr = prod_pool.tile([P, 512], f32, tag="pr")
                        nc.gpsimd.tensor_tensor(
                            out=pr[:].rearrange("p (r k) -> p r k", k=H),
                            in0=cp[:].rearrange("p (r k) -> p r k", k=H),
                            in1=tl3,
                            op=mybir.AluOpType.mult,
                        )
                        nc.vector.tensor_reduce(
                            out=out_sb[:, ts(j, RPC)],
                            in_=pr[:].rearrange("p (r k) -> p r k", k=H),
                            axis=mybir.AxisListType.X,
                            op=mybir.AluOpType.add,
                        )
                nc.sync.dma_start(out=out_ap[ts(i, P), :], in_=out_sb[:])

        if ILV:
            # tile-interleaved emission with a one-batch lag: B(b) tiles are
            # emitted right after A(b) finishes, so the consume engines chew
            # batch b while PE runs Phase A of batch b+1
            with (
                tc.tile_pool(name="xa", bufs=4) as xa_pool,
                tc.tile_pool(name="xt", bufs=3) as xt_pool,
                tc.tile_pool(name="ha", bufs=4) as ha_pool,
                tc.tile_pool(
                    name="psA",
                    bufs=(1 if PAIR else (3 if MERGEAF else 2)),
                    space="PSUM",
                ) as psA,
                tc.tile_pool(name="psF", bufs=(1 if PAIR else 2), space="PSUM") as psF,
                tc.tile_pool(name="gx", bufs=4) as gx_pool,
                tc.tile_pool(name="gb", bufs=4) as gb_pool,
                tc.tile_pool(name="hb", bufs=3) as hb_pool,
                tc.tile_pool(name="prod", bufs=4) as prod_pool,
                tc.tile_pool(name="ob", bufs=3) as ob_pool,
                tc.tile_pool(
                    name="psM",
                    bufs=(3 if PAIR else (5 if MERGEAF else 4)),
                    space="PSUM",
                ) as psM,
            ):
                TPB = S // P  # tiles per batch
                psF_eff = psA if MERGEAF else psF
                psT_eff = psA if HTA else psM
                for step in range(NTILES + TPB):
                    if step < NTILES:
                        emit_A(step, xa_pool, xt_pool, ha_pool, psA, psF_eff)
                    if step >= TPB:
                        emit_B(step - TPB, gx_pool, gb_pool, hb_pool,
                               prod_pool, ob_pool, psT_eff, psM)
        else:
            with (
                tc.tile_pool(name="xa", bufs=3) as xa_pool,
                tc.tile_pool(name="xt", bufs=2) as xt_pool,
                tc.tile_pool(name="ha", bufs=3) as ha_pool,
                tc.tile_pool(name="psA", bufs=2, space="PSUM") as psA,
                tc.tile_pool(name="psF", bufs=2, space="PSUM") as psF,
            ):
                for i in range(NTILES):
                    emit_A(i, xa_pool, xt_pool, ha_pool, psA, psF)
            # head_all DRAM writes must complete before the gathers read them
            tc.strict_bb_all_engine_barrier()
            with (
                tc.tile_pool(name="gx", bufs=3) as gx_pool,
                tc.tile_pool(name="gb", bufs=3) as gb_pool,
                tc.tile_pool(name="hb", bufs=2) as hb_pool,
                tc.tile_pool(name="prod", bufs=3) as prod_pool,
                tc.tile_pool(name="ob", bufs=3) as ob_pool,
                tc.tile_pool(name="psT", bufs=2, space="PSUM") as psT,
                tc.tile_pool(name="psM", bufs=6, space="PSUM") as psM,
            ):
                for i in range(NTILES):
                    emit_B(i, gx_pool, gb_pool, hb_pool, prod_pool, ob_pool,
                           psT, psM)

    nc.compile()
    return nc


def prep_inputs(x, head_id, root, Wh, bh, Wt, bt, kernel):
    """Host-side prep: shard over batch, precompute gather indices & root head."""
    x = np.asarray(x, dtype=np.float32)
    head_id = np.asarray(head_id)
    root = np.asarray(root, dtype=np.float32)
    Wh = np.asarray(Wh, dtype=np.float32)
    bh = np.asarray(bh, dtype=np.float32)
    Wt = np.asarray(Wt, dtype=np.float32)
    bt = np.asarray(bt, dtype=np.float32)
    kernel = np.asarray(kernel, dtype=np.float32)

    rooth = np.maximum(root @ Wh + bh, 0.0).astype(np.float32).reshape(1, H)
    shared = {
        "Wh": Wh,
        "Wt": Wt,
        "bh": bh.reshape(1, H).astype(np.float32),
        "bt": bt.reshape(1, H).astype(np.float32),
        "rooth": rooth,
        "kern": kernel,
    }
    in_maps = []
    for c in range(NCORES):
        bs = slice(c * BPC, (c + 1) * BPC)
        hid = head_id[bs].astype(np.int64)
        boff = (np.arange(BPC, dtype=np.int64) * TBL)[:, None]
        gidx = (hid + boff).reshape(TOK, 1).astype(np.int32)
        m = dict(shared)
        m["x"] = np.ascontiguousarray(x[bs].reshape(TOK, D))
        m["gidx"] = gidx
        in_maps.append(m)
    return in_maps


_NC_CACHE = {}


def _get_program(with_bias=True):
    key = ("nc", with_bias)
    if key not in _NC_CACHE:
        _NC_CACHE[key] = build_program(with_bias=with_bias)
    return _NC_CACHE[key]


def kernel(x, head_id, root, Wh, bh, Wt, bt, kernel):
    import time

    from concourse import bass_utils

    in_maps = prep_inputs(x, head_id, root, Wh, bh, Wt, bt, kernel)
    with_bias = bool(np.any(np.asarray(bh)) or np.any(np.asarray(bt)))
    nc = _get_program(with_bias=with_bias)
    res = None
    for attempt in range(6):
        try:
            res = bass_utils.run_bass_kernel_spmd(
                nc, in_maps, core_ids=list(range(NCORES))
            )
            break
        except Exception:
            # the first execution after a fresh NEFF compile (or right after
            # another session) occasionally fails at result fetch / hits a
            # transiently unrecoverable exec unit; the device recovers after
            # a short wait
            if attempt == 5:
                raise
            time.sleep(5.0 + 10.0 * attempt)
    outs = [res.results[c]["out"].reshape(BPC, S, R) for c in range(NCORES)]
    return np.concatenate(outs, axis=0)



# revision 19
# speedup vs baseline: 2.1261x; 2.1261x over previous
"""Trainium2 Bass kernel for nn_BERTSyntaxRel (biaffine syntax-relation head).

Computation (per batch b, token t):
    appended = concat([root, x[b]])                      # (S+1, D)
    gathered = appended[head_id[b, t]]                   # (D,)
    head = relu(gathered @ Wh + bh)                      # (H,)
    tail = relu(x[b, t] @ Wt + bt)                       # (H,)
    out[b, t, r] = sum_{h,k} head[h] * K[h, r, k] * tail[k]

Sharding: data-parallel over batch, 4 batches per core on 8 cores.

v2 design (everything bf16 into the PE; k-major biaffine):
- host pre-transposes x to xT [D, TOK] bf16; FF head branch is token-major
  (lhsT = xT chunks, rhs = Wh chunks -> hp[t,h]), tail branch feature-major
  (lhsT = Wt chunks, rhs = xT chunks -> tp[k,t]) so no PE transposes in
  phase A and tail lands in the layout the consume needs.
- per-batch head tables in DRAM (bf16 rows); batched indirect gather
  (8 tiles per SWDGE launch) + one PE transpose per tile -> headT [h,t].
- biaffine k-major: per r, U3_r[k,t] = sum_h K[h, r*H+k] * headT[h,t]
  (48 x 128-row matmuls per tile); elementwise mul with tailT on a per-group
  engine route (DVE direct from PSUM / ACT-copy + DVE 2x / gpsimd); the
  reduce over k runs on the PE as 1-row ones-matmuls into out columns.
"""

import os as _os

import numpy as np

B, S, D, H, R = 32, 1024, 768, 128, 48
NCORES = 8
BPC = B // NCORES            # batches per core (4)
TOK = BPC * S                # tokens per core (4096)
P = 128                      # partition dim / token tile
NTILES = TOK // P            # 32 token tiles per core
TPB = S // P                 # tiles per batch (8)
TBL = S + 1                  # rows per batch gather table (1025)
DC = D // P                  # 6 contraction chunks of 128
NG = 12                      # consume groups per tile (4 r's each)
RPG = R // NG                # r values per group (4)

# per-group mul route (gpsimd cannot read PSUM; DMA cannot read PSUM):
#   d = DVE mul direct from PSUM
#   a = ACT copy to bf16 SBUF + DVE 2x mul
#   g = ACT copy to bf16 SBUF + gpsimd mul
ROUTES = _os.environ.get("K_ROUTES", "dagddgadgddg")
GB = int(_os.environ.get("K_GB", "8"))       # tiles per indirect gather
XB = int(_os.environ.get("K_XB", "2"))       # tiles per xT load
OB = int(_os.environ.get("K_OB", "4"))       # tiles per output store
HB = int(_os.environ.get("K_HB", "2"))       # tiles per head-table write
LOOKAHEAD = int(_os.environ.get("K_LA", "1"))  # U3 groups emitted ahead of reduces
DEBUG = _os.environ.get("K_DEBUG", "0") == "1"  # dump intermediates to DRAM


def build_program(with_bias=False):
    """Build the Bass program (shared by all 8 cores, SPMD)."""
    from contextlib import ExitStack

    import concourse.bass as bass
    import concourse.tile as tile
    from concourse import bacc, mybir
    from concourse.masks import make_identity

    f32 = mybir.dt.float32
    bf16 = mybir.dt.bfloat16
    i32 = mybir.dt.int32
    ts = bass.ts

    nc = bacc.Bacc(
        "TRN2",
        target_bir_lowering=False,
        debug=False,
        num_devices=NCORES,
    )

    i16 = mybir.dt.int16
    GBT = 4                # tiles per dma_gather (>512 idxs crashes the DGE)
    GN = GBT * P           # idxs per gather (512)
    IDXC = GN // 16        # idx columns per gather group (32)
    NGRP = NTILES // GBT   # gather groups per core

    xT_ap = nc.dram_tensor("xT", [D, TOK], bf16, kind="ExternalInput").ap()
    gidx_ap = nc.dram_tensor("gidx", [P, NGRP * IDXC], i16,
                             kind="ExternalInput").ap()
    wh_ap = nc.dram_tensor("Whc", [P, DC * H], bf16, kind="ExternalInput").ap()
    wt_ap = nc.dram_tensor("Wtc", [P, DC * H], bf16, kind="ExternalInput").ap()
    bh_ap = nc.dram_tensor("bh", [1, H], bf16, kind="ExternalInput").ap()
    bt_ap = nc.dram_tensor("bt", [1, H], f32, kind="ExternalInput").ap()
    rooth_ap = nc.dram_tensor("rooth", [1, H], bf16, kind="ExternalInput").ap()
    kern_ap = nc.dram_tensor("kern", [H, R * H], bf16, kind="ExternalInput").ap()
    out_ap = nc.dram_tensor("out", [TOK, R], f32, kind="ExternalOutput").ap()
    if DEBUG:
        dbg_tail = nc.dram_tensor("dbg_tail", [P, TOK], bf16,
                                  kind="ExternalOutput").ap()
        dbg_head = nc.dram_tensor("dbg_head", [BPC * TBL, H], bf16,
                                  kind="ExternalOutput").ap()
        dbg_hTb = nc.dram_tensor("dbg_hTb", [P, S], bf16,
                                 kind="ExternalOutput").ap()
        dbg_prod = nc.dram_tensor("dbg_prod", [P, RPG * H], bf16,
                                  kind="ExternalOutput").ap()

    with tile.TileContext(nc) as tc, ExitStack() as ctx:
        # ---- constants / weights, resident for the whole kernel ----
        const = ctx.enter_context(tc.tile_pool(name="const", bufs=1))
        ident = const.tile([P, P], bf16)
        make_identity(nc, ident[:])
        ones_sb = const.tile([P, 1], bf16)
        nc.gpsimd.memset(ones_sb[:], 1.0)
        ones1 = const.tile([1, P], bf16)
        nc.gpsimd.memset(ones1[:], 1.0)

        whsb = const.tile([P, DC * H], bf16)   # [d-in-chunk, (c, h)]
        wtsb = const.tile([P, DC * H], bf16)   # [d-in-chunk, (c, k)]
        nc.sync.dma_start(out=whsb[:], in_=wh_ap[:])
        nc.sync.dma_start(out=wtsb[:], in_=wt_ap[:])
        bh_sb = const.tile([1, H], bf16)
        bt_sb = const.tile([P, 1], f32)
        rt_sb = const.tile([1, H], bf16)
        if with_bias:
            nc.sync.dma_start(out=bh_sb[:], in_=bh_ap[:])
            nc.sync.dma_start(out=bt_sb[:], in_=bt_ap.rearrange("o k -> k o"))
        nc.sync.dma_start(out=rt_sb[:], in_=rooth_ap[:])
        ksb = const.tile([H, R * H], bf16)     # 12KB/partition
        nc.sync.dma_start(out=ksb[:], in_=kern_ap[:])

        # tail states, feature-major, whole core resident: [k, tok] 8KB/part
        tailT_all = const.tile([P, TOK], bf16)
        gidx_sb = const.tile([P, NGRP * IDXC], i16)
        nc.sync.dma_start(out=gidx_sb[:], in_=gidx_ap[:])

        # per-batch gather tables in DRAM: row b*TBL is the root head state
        dram = ctx.enter_context(tc.tile_pool(name="dram", bufs=1, space="DRAM"))
        head_all = dram.tile([BPC * TBL, H], bf16)
        tbl_writes = [[] for _ in range(BPC)]
        for b in range(BPC):
            w = nc.sync.dma_start(
                out=head_all[b * TBL : b * TBL + 1, :], in_=rt_sb[:1, :]
            )
            tbl_writes[b].append(w.ins)

        xT4 = xT_ap.rearrange("(c p) t -> p c t", p=P)  # [128, 6, TOK]

        with (
            tc.tile_pool(name="xa", bufs=2) as xa_pool,
            tc.tile_pool(name="ha", bufs=2) as ha_pool,
            tc.tile_pool(name="gb", bufs=2) as gb_pool,
            tc.tile_pool(name="prod", bufs=4) as prod_pool,
            tc.tile_pool(name="ob", bufs=2) as ob_pool,
            tc.tile_pool(name="psFF", bufs=2, space="PSUM") as psFF,
            tc.tile_pool(name="psU3", bufs=4, space="PSUM") as psU3,
            tc.tile_pool(name="psO", bufs=2, space="PSUM") as psO,
        ):
            state = {}

            def emit_A(i):
                b = i // TPB
                if i % XB == 0:
                    xt2 = xa_pool.tile([P, DC * XB * P], bf16, name="xt2")
                    nc.sync.dma_start(
                        out=xt2[:].rearrange("p (c t) -> p c t", c=DC),
                        in_=xT4[:, :, i * P : (i + XB) * P],
                    )
                    state["xt2"] = xt2
                xt2 = state["xt2"]
                off = (i % XB) * P

                def xsl(c):
                    return xt2[:, c * XB * P + off : c * XB * P + off + P]

                ps = psFF.tile([P, 2 * H], f32)
                for c in range(DC):
                    nc.tensor.matmul(
                        out=ps[:, :H], lhsT=xsl(c), rhs=whsb[:, ts(c, H)],
                        start=(c == 0), stop=(c == DC - 1 and not with_bias),
                    )
                if with_bias:
                    nc.tensor.matmul(
                        out=ps[:, :H], lhsT=ones1[:1, :], rhs=bh_sb[:1, :],
                        start=False, stop=True,
                    )
                for c in range(DC):
                    nc.tensor.matmul(
                        out=ps[:, H:], lhsT=wtsb[:, ts(c, H)], rhs=xsl(c),
                        start=(c == 0), stop=(c == DC - 1),
                    )
                if i % HB == 0:
                    state["ha2"] = ha_pool.tile([P, HB * H], bf16, name="ha2")
                ha2 = state["ha2"]
                nc.scalar.activation(
                    out=ha2[:, ts(i % HB, H)], in_=ps[:, :H],
                    func=mybir.ActivationFunctionType.Relu,
                )
                nc.scalar.activation(
                    out=tailT_all[:, ts(i, P)], in_=ps[:, H:],
                    func=mybir.ActivationFunctionType.Relu,
                    bias=bt_sb[:] if with_bias else 0.0,
                )
                if i % HB == HB - 1:
                    row0 = b * TBL + 1 + ((i % TPB) - (HB - 1)) * P
                    w = nc.sync.dma_start(
                        out=head_all[row0 : row0 + HB * P, :].rearrange(
                            "(j t) h -> t j h", j=HB
                        ),
                        in_=ha2[:].rearrange("t (j h) -> t j h", j=HB),
                    )
                    tbl_writes[b].append(w.ins)

            def emit_B(i):
                b = i // TPB
                if i % GBT == 0:
                    # transposing gather: headT[h, t] for GBT tiles of tokens
                    grp = i // GBT
                    hTb = gb_pool.tile([P, GN], bf16, name="hTb")
                    g = nc.gpsimd.dma_gather(
                        out_ap=hTb[:].rearrange("p (o t) -> p o t", o=1),
                        in_ap=head_all[:],
                        idxs_ap=gidx_sb[:, grp * IDXC : (grp + 1) * IDXC],
                        num_idxs=GN,
                        num_idxs_reg=GN,
                        elem_size=H,
                        transpose=True,
                    )
                    for w_ins in tbl_writes[b]:
                        tile.add_dep_helper(g.ins, w_ins, sync=True,
                                            reason="head_all RAW")
                    state["hTb"] = hTb
                headT = state["hTb"][:, ts(i % GBT, P)]

                pso = psO.tile([P, R], f32)
                tl_b = (
                    tailT_all[:, ts(i, P)]
                    .rearrange("p (o k) -> p o k", o=1)
                    .to_broadcast([P, RPG, H])
                )

                pend = []  # (prod, g) waiting for their reduce matmuls

                def flush_reduce():
                    prod, g = pend.pop(0)
                    for q in range(RPG):
                        nc.tensor.matmul(
                            out=pso[:, g * RPG + q : g * RPG + q + 1],
                            lhsT=prod[:, ts(q, H)], rhs=ones_sb[:, :1],
                            start=True, stop=True,
                        )

                for g in range(NG):
                    u3 = psU3.tile([P, RPG * H], f32)
                    for q in range(RPG):
                        nc.tensor.matmul(
                            out=u3[:, ts(q, H)],
                            lhsT=ksb[:, ts(g * RPG + q, H)], rhs=headT,
                            start=True, stop=True,
                        )
                    route = ROUTES[g % len(ROUTES)]
                    prod = prod_pool.tile([P, RPG * H], bf16, tag="pr")
                    u3v = u3[:].rearrange("p (r k) -> p r k", k=H)
                    prodv = prod[:].rearrange("p (r k) -> p r k", k=H)
                    if route == "d":
                        nc.vector.tensor_tensor(
                            out=prodv, in0=u3v, in1=tl_b, op=mybir.AluOpType.mult
                        )
                    else:
                        cp = prod_pool.tile([P, RPG * H], bf16, tag="cp")
                        nc.scalar.copy(out=cp[:], in_=u3[:])
                        eng = nc.gpsimd if route == "g" else nc.vector
                        eng.tensor_tensor(
                            out=prodv,
                            in0=cp[:].rearrange("p (r k) -> p r k", k=H),
                            in1=tl_b, op=mybir.AluOpType.mult,
                        )
                    if DEBUG and i == 0 and g == 0:
                        state["dbg_prod"] = prod
                    pend.append((prod, g))
                    if len(pend) > LOOKAHEAD:
                        flush_reduce()
                while pend:
                    flush_reduce()
                if DEBUG and i == 0:
                    state["dbg_hTb"] = state["hTb"]

                if i % OB == 0:
                    state["ob"] = ob_pool.tile([P, OB * R], f32, name="ob")
                ob = state["ob"]
                nc.scalar.copy(out=ob[:, ts(i % OB, R)], in_=pso[:])
                if i % OB == OB - 1:
                    nc.sync.dma_start(
                        out=out_ap[(i - (OB - 1)) * P : (i + 1) * P, :].rearrange(
                            "(j t) r -> t j r", j=OB
                        ),
                        in_=ob[:].rearrange("t (j r) -> t j r", j=OB),
                    )

            for step in range(NTILES + TPB):
                if step < NTILES:
                    emit_A(step)
                if step >= TPB:
                    emit_B(step - TPB)

            if DEBUG:
                nc.sync.dma_start(out=dbg_tail[:], in_=tailT_all[:])
                w = nc.sync.dma_start(out=dbg_head[:], in_=head_all[:])
                for b in range(BPC):
                    for w_ins in tbl_writes[b]:
                        tile.add_dep_helper(w.ins, w_ins, sync=True,
                                            reason="head_all dump RAW")
                nc.sync.dma_start(out=dbg_hTb[:], in_=state["dbg_hTb"][:])
                nc.sync.dma_start(out=dbg_prod[:], in_=state["dbg_prod"][:])

    nc.compile()
    return nc


def prep_inputs(x, head_id, root, Wh, bh, Wt, bt, kernel):
    """Host-side prep: shard over batch, transpose+cast x, gather indices."""
    import ml_dtypes

    bf16 = ml_dtypes.bfloat16

    x = np.asarray(x, dtype=np.float32)
    head_id = np.asarray(head_id)
    root = np.asarray(root, dtype=np.float32)
    Wh = np.asarray(Wh, dtype=np.float32)
    bh = np.asarray(bh, dtype=np.float32)
    Wt = np.asarray(Wt, dtype=np.float32)
    bt = np.asarray(bt, dtype=np.float32)
    kernel = np.asarray(kernel, dtype=np.float32)

    rooth = np.maximum(root @ Wh + bh, 0.0).astype(bf16).reshape(1, H)
    # weight chunks: [d-in-chunk, (c, h)] so chunk c is a [128, 128] free slice
    whc = np.ascontiguousarray(
        Wh.reshape(DC, P, H).transpose(1, 0, 2).reshape(P, DC * H)
    ).astype(bf16)
    wtc = np.ascontiguousarray(
        Wt.reshape(DC, P, H).transpose(1, 0, 2).reshape(P, DC * H)
    ).astype(bf16)
    shared = {
        "Whc": whc,
        "Wtc": wtc,
        "bh": bh.reshape(1, H).astype(bf16),
        "bt": bt.reshape(1, H).astype(np.float32),
        "rooth": rooth,
        "kern": kernel.astype(bf16),
    }
    GBT, GN = 4, 4 * P
    IDXC = GN // 16
    NGRP = NTILES // GBT
    in_maps = []
    for c in range(NCORES):
        bs = slice(c * BPC, (c + 1) * BPC)
        hid = head_id[bs].astype(np.int64)           # (BPC, S)
        boff = (np.arange(BPC, dtype=np.int64) * TBL)[:, None]
        gidx = (hid + boff).astype(np.int16).reshape(TOK)
        # dma_gather idx layout: idx i of group g at [i % 16, g*IDXC + i // 16],
        # replicated into all eight 16-partition stripes (one per Q7 core)
        gidx_w = np.zeros((P, NGRP * IDXC), np.int16)
        for g in range(NGRP):
            blk = gidx[g * GN : (g + 1) * GN].reshape(IDXC, 16).T
            gidx_w[:, g * IDXC : (g + 1) * IDXC] = np.tile(blk, (8, 1))
        m = dict(shared)
        m["xT"] = np.ascontiguousarray(
            x[bs].reshape(TOK, D).T
        ).astype(bf16)
        m["gidx"] = gidx_w
        in_maps.append(m)
    return in_maps


_NC_CACHE = {}


def _get_program(with_bias=False):
    key = ("nc", with_bias)
    if key not in _NC_CACHE:
        _NC_CACHE[key] = build_program(with_bias=with_bias)
    return _NC_CACHE[key]


def kernel(x, head_id, root, Wh, bh, Wt, bt, kernel):
    import time

    from concourse import bass_utils

    in_maps = prep_inputs(x, head_id, root, Wh, bh, Wt, bt, kernel)
    with_bias = bool(np.any(np.asarray(bh)) or np.any(np.asarray(bt)))
    nc = _get_program(with_bias=with_bias)
    res = None
    for attempt in range(6):
        try:
            res = bass_utils.run_bass_kernel_spmd(
                nc, in_maps, core_ids=list(range(NCORES))
            )
            break
        except Exception:
            # the first execution after a fresh NEFF compile occasionally
            # fails transiently; the device recovers after a short wait
            if attempt == 5:
                raise
            time.sleep(5.0 + 10.0 * attempt)
    outs = [res.results[c]["out"].reshape(BPC, S, R) for c in range(NCORES)]
    return np.concatenate(outs, axis=0)
